# revision 2
# baseline (speedup 1.0000x reference)
"""TRN2 Bass kernel for nn_MAD_4612794876395 (retrieval_knn).

Math: with dist = softmax_k(-||pos_d - pos_r||) and sum_k dist = 1, the
reference output collapses to
    out[b,c] = wmem@adapt_w + adapt_b + wdiff@field_b.reshape(H,C)
             + sum_h wdiff[b,h] * (date@field_w)[b, h*C+c]
where wdiff[b,h] = sum_k dist[b,k]*diff[b,k,h].  The dominant term is the
137 GFLOP date@field_w product, computed on 8 NeuronCores tensor-parallel
over field_w's 65536 columns (64 h-values per core) as fp16 matmuls at
1 row/cycle.

The h-contraction (one multiply-add per matmul output element) is the
hard part: per-partition-scalar ops force 128-wide tiles and the three
elementwise engines together cannot sustain 4 such ops per 853ns matmul
chain.  Instead field_w columns are reordered c-major/h-minor on the
host so each PSUM tile is [128b, 8c x 64h], and a custom DVE op
(out = running sum of in0*in1, one elem/cycle) computes weighted prefix
sums in a single 512-wide pass; the 8 segment ends are DMA-extracted
and the host finishes with a cheap difference.  Small terms are host
numpy.

v2 scheduling: the warm matmul stream already runs at the 216 ns/MM
hardware floor, so the wins are all at the edges:
 - startup DMA triggers cost ~650 ns each on the serialized Sync queue,
   which is busy with framework preamble until ~7.3 us.  The Activation
   queue (also hardware-DGE) is free at ~6.0 us and otherwise unused, so
   the startup-critical loads go there as a few large partition-major
   packed transfers ordered by consumption priority.
 - fw slice prefetch is one 512 KB trigger per slice (4 KB packets)
   on the Sync queue instead of 4x128 KB.
 - the PE clock is HAM-throttled to 1.2 GHz until ~3.4 us of sustained
   activity: a run of dummy matmuls on a zeroed tile spans the DMA wait
   so the real matmuls start at full 2.4 GHz.
"""
import sys

sys.path.insert(0, "/opt/trn_rl_repo")

import numpy as np

N_DATA, F, H, C, K, B = 100000, 512, 512, 128, 8, 2048
NCORES = 8
HSH = H // NCORES          # 64 h-values per core
SH = HSH * C               # 8192 field_w cols per core
P = 128
NB = B // P                # 16 b-tiles
NS = SH // 512             # 16 col-slices of 512 (8 c-segments x 64 h)
N_DUMMY = 12               # HAM warmup matmuls (N=256, ~0.3 us each cold)

_NC = None
_LAST_IN_MAPS = None

_WSUM_SHAS = {"v3": "b3fc3e78a862b7eb", "v4": "bc6a002865d48b97"}


def _register_wsum():
    """Register the weighted-prefix-sum custom DVE op (idempotent)."""
    from concourse import dve_ops
    from concourse.dve_spec import Spec, Src0, Src1, scan, AluOp

    name = "ANT_WSUM_SCAN"
    for op in dve_ops.OPS:
        if op.name == name:
            return op

    def ref(in0, in1, s0, s1, imm2):
        p0 = in0.astype(np.float32).reshape(in0.shape[0], -1)
        p1 = in1.astype(np.float32).reshape(in1.shape[0], -1)
        return np.cumsum(p0 * p1, axis=-1).reshape(in0.shape)

    spec = Spec(body=scan(AluOp.ADD, Src0 * Src1), reference=ref)
    op = dve_ops.DveOp(name, spec, subdim=False, uops_sha=dict(_WSUM_SHAS))
    dve_ops.OPS.append(op)
    dve_ops._SUB_OPCODE_FOR_NAME[name] = (
        max(dve_ops._SUB_OPCODE_FOR_NAME.values()) + 1)
    assert dve_ops._SUB_OPCODE_FOR_NAME[name] < 0x20
    return op


def _build():
    import concourse.bass as bass
    import concourse.mybir as mybir
    import concourse.tile as tile
    from concourse import bacc

    wsum = _register_wsum()

    nc = bacc.Bacc(None, target_bir_lowering=False, debug=False)
    # dateT partition-major: d0[p, fc, j] = date[j, fc*128+p] (b-cols 0:512),
    # d1[p, fc, j] = date[512+j, fc*128+p] (b-cols 512:2048)
    d0 = nc.dram_tensor("d0", [P, 4, 512], mybir.dt.float16,
                        kind="ExternalInput")
    d1 = nc.dram_tensor("d1", [P, 4, 1536], mybir.dt.float16,
                        kind="ExternalInput")
    # fw slice 0 partition-major: fw0[p, fc*512+c] = fw2[fc*128+p, c]
    fw0 = nc.dram_tensor("fw0", [P, 4 * 512], mybir.dt.float16,
                         kind="ExternalInput")
    # fw slices 1..15: fwn[(n-1)*128+p, fc*512+c] = fw2[fc*128+p, n*512+c]
    fwn = nc.dram_tensor("fwn", [(NS - 1) * P, 4 * 512], mybir.dt.float16,
                         kind="ExternalInput")
    # wdiff b-tiled: [p, t, h] = wdiff[t*128+p, h] -> contiguous col-splits
    wds = nc.dram_tensor("wds", [P, NB * HSH], mybir.dt.float32,
                         kind="ExternalInput")
    # prefix-sum segment ends; host differences them into per-c sums
    ends = nc.dram_tensor("ends", [B, C], mybir.dt.float32,
                          kind="ExternalOutput")

    with tile.TileContext(nc) as tc:
        with (
            tc.tile_pool(name="const", bufs=1) as cp,
            tc.tile_pool(name="fwp", bufs=6) as fwp,
            tc.tile_pool(name="wgp", bufs=8) as wgp,
            tc.tile_pool(name="ps", bufs=7, space="PSUM") as ps,
            tc.tile_pool(name="dps", bufs=1, space="PSUM") as dps,
        ):
            # resident fp16 dateT, wdiff rows, SBUF staging for segment ends
            dall = cp.tile([P, 4, B], mybir.dt.float16, name="dall")
            f0 = cp.tile([P, 4 * 512], mybir.dt.float16, name="f0")
            wrall = cp.tile([P, NB, HSH], mybir.dt.float32, name="wrall")
            es = [cp.tile([P, C], mybir.dt.float32, name=f"es{t}")
                  for t in range(NB)]
            dum = cp.tile([P, 256], mybir.dt.float16, name="dum")
            dum_ps = dps.tile([P, 256], mybir.dt.float32, name="dum_ps")

            # per-b-tile wdiff rows repeated 8x via a stride-0 middle dim
            wrb = []
            for t in range(NB):
                s = wrall[:, t:t + 1, :]
                wrb.append(bass.AP(s.tensor, s.offset,
                                   [s.ap[0], [0, 8], s.ap[-1]]))

            # HAM warmup: dummy matmuls on a zeroed tile span the startup
            # DMA wait so real matmuls begin at the full 2.4 GHz clock
            nc.gpsimd.memset(dum[:], 0.0)
            for _ in range(N_DUMMY):
                nc.tensor.matmul(dum_ps[:], dum[:, 0:128], dum[:],
                                 start=True, stop=True)

            # startup loads on the Activation HWDGE queue (free ~1.3 us
            # before Sync), largest-consumer-first
            nc.scalar.dma_start(f0[:, 0:512], fw0[:, 0:512])
            nc.scalar.dma_start(dall[:, :, 0:512], d0[:])
            nc.scalar.dma_start(f0[:, 512:2048], fw0[:, 512:2048])
            nc.scalar.dma_start(wrall[:, 0:4, :], wds[:, 0:4 * HSH])
            nc.scalar.dma_start(dall[:, :, 512:1280], d1[:, :, 0:768])
            nc.scalar.dma_start(dall[:, :, 1280:2048], d1[:, :, 768:1536])
            nc.scalar.dma_start(wrall[:, 4:NB, :], wds[:, 4 * HSH:])

            # fw slice prefetch on the Sync queue, one trigger per slice
            fts = [f0]
            for n in range(1, NS):
                ft = fwp.tile([P, 4 * 512], mybir.dt.float16, name="ft",
                              tag="ft")
                nc.sync.dma_start(ft[:], fwn[(n - 1) * P:n * P, :])
                fts.append(ft)

            for n in range(NS):
                for t in range(NB):
                    g = ps.tile([P, 512], mybir.dt.float32, name="g", tag="g")
                    for fc in range(4):
                        nc.tensor.matmul(g[:],
                                         dall[:, fc, t * P:(t + 1) * P],
                                         fts[n][:, fc * 512:(fc + 1) * 512],
                                         start=(fc == 0), stop=(fc == 3))
                    # weighted prefix sum over the tile in one DVE pass
                    wg = wgp.tile([P, 8, HSH], mybir.dt.float32, name="wg",
                                  tag="wg")
                    nc.vector._custom_dve(wsum, out=wg[:], in0=g[:],
                                          in1=wrb[t])
                    # stage segment ends in SBUF (GPSIMD is otherwise idle)
                    nc.gpsimd.tensor_copy(es[t][:, n * 8:(n + 1) * 8],
                                          wg[:, :, HSH - 1:HSH])
                    if n == NS - 1:
                        nc.sync.dma_start(ends[t * P:(t + 1) * P, :],
                                          es[t][:])
    nc.finalize()
    return nc


def kernel(idx, date, train_dates, mem, train_nns, pos_w, pos_b, field_w,
           field_b, adapt_w, adapt_b):
    global _NC, _LAST_IN_MAPS
    from concourse.bass_utils import run_bass_kernel_spmd

    idx = np.asarray(idx)
    date = np.asarray(date, dtype=np.float32)
    train_dates = np.asarray(train_dates, dtype=np.float32)
    mem = np.asarray(mem, dtype=np.float32)
    train_nns = np.asarray(train_nns)
    pos_w = np.asarray(pos_w, dtype=np.float32)
    pos_b = np.asarray(pos_b, dtype=np.float32)
    field_w = np.asarray(field_w, dtype=np.float32)
    field_b = np.asarray(field_b, dtype=np.float32)
    adapt_w = np.asarray(adapt_w, dtype=np.float32)
    adapt_b = np.asarray(adapt_b, dtype=np.float32)

    # ---- host phase 1 (small): dist, wdiff, const terms ----
    refs = train_nns[idx]                                   # [B, K]
    pos_d = date @ pos_w + pos_b                            # [B, H]
    pos_r = (train_dates[refs.reshape(-1)] @ pos_w + pos_b).reshape(B, K, H)
    diff = pos_d[:, None, :] - pos_r                        # [B, K, H]
    norm = np.sqrt((diff * diff).sum(-1))                   # [B, K]
    m = norm.min(axis=1, keepdims=True)
    e = np.exp(m - norm)
    dist = e / e.sum(axis=1, keepdims=True)                 # [B, K]
    wdiff = np.einsum("bk,bkh->bh", dist, diff).astype(np.float32)
    wmem = np.einsum("bk,bkc->bc", dist, mem[refs]).astype(np.float32)
    const = wmem @ adapt_w + adapt_b + wdiff @ field_b.reshape(H, C)

    # ---- device phase 2: grad-term, TP over the 65536 dim ----
    if _NC is None:
        _NC = _build()
    dateT16 = date.T.astype(np.float16)                     # [F, B]
    date3 = dateT16.reshape(4, P, B)                        # [fc, p, b]
    d0 = np.ascontiguousarray(date3[:, :, 0:512].transpose(1, 0, 2))
    d1 = np.ascontiguousarray(date3[:, :, 512:2048].transpose(1, 0, 2))
    fw3 = field_w.reshape(F, H, C)                          # [f, h, c]
    in_maps = []
    for i in range(NCORES):
        # c-major/h-minor columns for this core's h range
        fw2 = np.ascontiguousarray(
            fw3[:, i * HSH:(i + 1) * HSH, :].transpose(0, 2, 1)
        ).reshape(F, SH).astype(np.float16)                 # col = c*64 + h
        fw4 = fw2.reshape(4, P, NS, 512)                    # [fc, p, n, c]
        fw0 = np.ascontiguousarray(
            fw4[:, :, 0, :].transpose(1, 0, 2)).reshape(P, 4 * 512)
        fwn = np.ascontiguousarray(
            fw4[:, :, 1:, :].transpose(2, 1, 0, 3)).reshape((NS - 1) * P,
                                                            4 * 512)
        # wdiff b-tiled [p, t, h] = wdiff[t*128+p, i*64+h]
        wdt = np.ascontiguousarray(
            wdiff[:, i * HSH:(i + 1) * HSH]
            .reshape(NB, P, HSH).transpose(1, 0, 2).reshape(P, NB * HSH))
        in_maps.append({
            "d0": d0.reshape(P, 4, 512),
            "d1": d1.reshape(P, 4, 1536),
            "fw0": fw0,
            "fwn": fwn,
            "wds": wdt,
        })
    _LAST_IN_MAPS = in_maps
    res = run_bass_kernel_spmd(_NC, in_maps, core_ids=list(range(NCORES)))
    grad_term = np.zeros((B, C), dtype=np.float32)
    for i in range(NCORES):
        e8 = res.results[i]["ends"].reshape(B, NS, 8)
        grad_term += np.diff(e8, axis=2, prepend=0.0).reshape(B, C)
    return (const + grad_term).astype(np.float32)


def run_device(trace=False):
    """Re-run the device phase on the last inputs (test.py profiling)."""
    from concourse.bass_utils import run_bass_kernel_spmd
    assert _NC is not None and _LAST_IN_MAPS is not None
    return run_bass_kernel_spmd(_NC, _LAST_IN_MAPS,
                                core_ids=list(range(NCORES)), trace=trace)
